# revision 33
# baseline (speedup 1.0000x reference)
"""GAT (2-layer, 2-head) + MLP head on 8 Trainium2 NeuronCores.

Strategy (graph/data parallel, per sharding hint):
  - Nodes sharded 1250/core; edges partitioned by dst and sorted by dst on
    the host, grouped into dst-blocks of 125 rows, edge chunks of 128.
  - Attention logits fold into extra GEMM columns on the host:
    a_s = x @ (W_s . att_s), a_d = x @ (W_d . att_d), so xd is never built.
  - Per-layer per-core: local GEMM -> AllGather of [N, row] bf16 feature
    rows (msg cols + fp32 a-cols packed in-row) -> dma_gather per edge of
    a-rows (src & dst) and msg rows (per head) -> p = exp(leaky(a_s+a_d))
    -> segment aggregation as chunked mask matmuls on the PE
    (mask scaled per-edge by p via ACT; denominator via raw-mask matmul)
    -> normalize by 1/denom, bias, relu.
  - Head: t = h2 @ wf1 + bf1 per block, batchnorm stats via ones-matmul +
    AllReduce, normalize + relu + wf2 + log_softmax.

kernel(**inputs) takes FULL inputs, returns FULL [10000, 3] fp32 output.
"""
import sys

sys.path.insert(0, "/opt/trn_rl_repo")

import numpy as np
import ml_dtypes

import concourse.bass as bass
import concourse.bacc as bacc
import concourse.mybir as mybir
import concourse.tile as tile
from concourse.bass_utils import run_bass_kernel_spmd

F32 = mybir.dt.float32
BF16 = mybir.dt.bfloat16
I16 = mybir.dt.int16

N = 10000
E = 160000
H = 2
D_IN = 256
C1 = 256          # per-head channels conv1 -> msg row 512
C2 = 512          # per-head channels conv2 -> msg row 1024
OUT = 3
NCORES = 8
NLOC = N // NCORES          # 1250
DBLK = 125                  # dst rows per block
NBLK = NLOC // DBLK         # 10

ROW1 = 640                  # bf16 cols: 512 msg | 4xf32 a (cols 512..519) | pad
ROW2 = 1152                 # bf16 cols: 1024 msg | 4xf32 a (1024..1031) | pad
GMAX = 8                    # max 128-chunks per dma_gather (1024 idxs)

bf = lambda a: np.asarray(a, ml_dtypes.bfloat16)


# ----------------------------------------------------------------- host prep

def _wrap_tab(idx, epad):
    """dma_gather index table: [128, epad/16] int16, token s at [s%16, s//16],
    replicated across the 8 groups of 16 partitions."""
    t = idx.astype(np.int16).reshape(epad // 16, 16).T        # [16, epad/16]
    return np.ascontiguousarray(np.tile(t, (8, 1)))


def _prep_graph(edge_index):
    src = np.concatenate([np.asarray(edge_index[0]), np.arange(N)]).astype(np.int64)
    dst = np.concatenate([np.asarray(edge_index[1]), np.arange(N)]).astype(np.int64)
    order = np.argsort(dst, kind="stable")
    src, dst = src[order], dst[order]
    core = dst // NLOC
    blk = (dst % NLOC) // DBLK
    counts = np.zeros((NCORES, NBLK), np.int64)
    np.add.at(counts, (core, blk), 1)
    nch = [max(1, int(-(-counts[:, b].max() // 128))) for b in range(NBLK)]
    totch = sum(nch)
    epad = totch * 128
    ch_off = np.concatenate([[0], np.cumsum(nch)]).astype(np.int64)

    src_tabs, dst_tabs, masks = [], [], []
    for c in range(NCORES):
        s_pad = np.zeros(epad, np.int64)
        d_pad = np.zeros(epad, np.int64)
        mask = np.zeros((128, totch * DBLK), np.float32)
        for b in range(NBLK):
            sel = (core == c) & (blk == b)
            ss, dd = src[sel], dst[sel]
            cnt = ss.shape[0]
            base = int(ch_off[b]) * 128
            s_pad[base:base + cnt] = ss
            d_pad[base:base + cnt] = dd
            j = np.arange(cnt)
            kk = int(ch_off[b]) + j // 128
            pp = j % 128
            dloc = (dd % NLOC) % DBLK
            mask[pp, kk * DBLK + dloc] = 1.0
        src_tabs.append(_wrap_tab(s_pad, epad))
        dst_tabs.append(_wrap_tab(d_pad, epad))
        masks.append(bf(mask))
    return nch, ch_off, epad, src_tabs, dst_tabs, masks


def _fold_att(W, att, C):
    # v[k, h] = sum_c W[k, h*C + c] * att[h, c]
    K = W.shape[0]
    return np.stack([W[:, h * C:(h + 1) * C] @ att[h] for h in range(H)], 1)


# ------------------------------------------------------------- kernel build

def build(nch, ch_off, phases="full"):
    totch = sum(nch)
    epad = totch * 128
    nc = bacc.Bacc("TRN2", target_bir_lowering=False, debug=False,
                   num_devices=NCORES)

    def din(name, shape, dt):
        return nc.dram_tensor(name, shape, dt, kind="ExternalInput")

    xT_d = din("xT", [D_IN, NLOC], BF16)
    w1_d = din("w1aug", [D_IN, 516], BF16)
    w2_d = din("w2aug", [4 * 128, 1028], BF16)
    wf1_d = din("wf1", [8 * 128, 128], BF16)
    wf2_d = din("wf2", [128, OUT], BF16)
    b1r_d = din("b1rep", [128, 512], F32)
    b2r_d = din("b2rep", [128, 1024], F32)
    bf1r_d = din("bf1rep", [128, 128], F32)
    bf2r_d = din("bf2rep", [128, OUT], F32)
    gb_d = din("gb", [2, 128], F32)          # gamma, beta
    ident_d = din("ident", [128, 128], BF16)
    ones_d = din("ones", [128, 128], F32)
    stab_d = din("src_tab", [128, epad // 16], I16)
    dtab_d = din("dst_tab", [128, epad // 16], I16)
    mask_d = din("mask", [128, totch * DBLK], BF16)
    out_d = nc.dram_tensor("out", [NLOC, OUT], F32, kind="ExternalOutput")

    def body(tc):
        with tc.tile_pool(name="const", bufs=1) as cp, \
             tc.tile_pool(name="dram", bufs=1, space="DRAM") as dram:
            # ---- resident SBUF constants
            xT = cp.tile([128, 2, NLOC], BF16)
            for kc in range(2):
                nc.sync.dma_start(out=xT[:, kc, :], in_=xT_d[kc * 128:(kc + 1) * 128, :])
            w1 = cp.tile([128, 2, 516], BF16)
            for kc in range(2):
                nc.sync.dma_start(out=w1[:, kc, :], in_=w1_d[kc * 128:(kc + 1) * 128, :])
            w2 = cp.tile([128, 4, 1028], BF16)
            for kc in range(4):
                nc.sync.dma_start(out=w2[:, kc, :], in_=w2_d[kc * 128:(kc + 1) * 128, :])
            wf1 = cp.tile([128, 8, 128], BF16)
            for kc in range(8):
                nc.sync.dma_start(out=wf1[:, kc, :], in_=wf1_d[kc * 128:(kc + 1) * 128, :])
            wf2 = cp.tile([128, OUT], BF16)
            nc.sync.dma_start(out=wf2[:], in_=wf2_d[:])
            b1r = cp.tile([128, 512], F32)
            nc.sync.dma_start(out=b1r[:], in_=b1r_d[:])
            b2r = cp.tile([128, 1024], F32)
            nc.sync.dma_start(out=b2r[:], in_=b2r_d[:])
            bf1r = cp.tile([128, 128], F32)
            nc.sync.dma_start(out=bf1r[:], in_=bf1r_d[:])
            bf2r = cp.tile([128, OUT], F32)
            nc.sync.dma_start(out=bf2r[:], in_=bf2r_d[:])
            gam = cp.tile([1, 128], F32)
            nc.sync.dma_start(out=gam[:], in_=gb_d[0:1, :])
            bet = cp.tile([1, 128], F32)
            nc.sync.dma_start(out=bet[:], in_=gb_d[1:2, :])
            ident = cp.tile([128, 128], BF16)
            nc.sync.dma_start(out=ident[:], in_=ident_d[:])
            ones = cp.tile([128, 128], F32)
            nc.sync.dma_start(out=ones[:], in_=ones_d[:])
            stab = cp.tile([128, epad // 16], I16)
            nc.sync.dma_start(out=stab[:], in_=stab_d[:])
            dtab = cp.tile([128, epad // 16], I16)
            nc.sync.dma_start(out=dtab[:], in_=dtab_d[:])
            maskt = cp.tile([128, totch * DBLK], BF16)
            nc.sync.dma_start(out=maskt[:], in_=mask_d[:])

            xs1_my = dram.tile([NLOC, ROW1], BF16)
            xs1_full = dram.tile([N, ROW1], BF16)
            xs2_my = dram.tile([NLOC, ROW2], BF16)
            xs2_full = dram.tile([N, ROW2], BF16)
            st_my = dram.tile([1, 256], F32)
            st_all = dram.tile([1, 256], F32)

            # ================= P0: layer-1 local GEMM =================
            with tc.tile_pool(name="p0s", bufs=3) as sp, \
                 tc.tile_pool(name="p0p", bufs=2, space="PSUM") as pp:
                for b in range(NBLK):
                    ph0 = pp.tile([DBLK, 258], F32, tag="ph0")
                    ph1 = pp.tile([DBLK, 258], F32, tag="ph1")
                    for kc in range(2):
                        lhs = xT[:, kc, b * DBLK:(b + 1) * DBLK]
                        nc.tensor.matmul(ph0[:], lhs, w1[:, kc, 0:258],
                                         start=(kc == 0), stop=(kc == 1))
                        nc.tensor.matmul(ph1[:], lhs, w1[:, kc, 258:516],
                                         start=(kc == 0), stop=(kc == 1))
                    stg = sp.tile([DBLK, ROW1], BF16, tag="stg")
                    nc.vector.memset(stg[:, 520:ROW1], 0.0)
                    nc.vector.tensor_copy(stg[:, 0:258], ph0[:])
                    nc.vector.tensor_copy(stg[:, 258:512], ph1[:, 0:254])
                    stgf = stg[:].bitcast(F32)          # [DBLK, 320]
                    nc.vector.tensor_copy(stgf[:, 256:260], ph1[:, 254:258])
                    nc.sync.dma_start(out=xs1_my[b * DBLK:(b + 1) * DBLK, :],
                                      in_=stg[:])

            if phases == "p0":
                _dummy_out(nc, tc, out_d)
                return
            nc.gpsimd.collective_compute(
                "AllGather", mybir.AluOpType.bypass,
                replica_groups=[list(range(NCORES))],
                ins=[xs1_my.opt()], outs=[xs1_full.opt()])
            if phases == "ag1":
                _dummy_out(nc, tc, out_d)
                return

            # ============ P1: layer-1 edges + layer-2 input GEMM ============
            emode = {"p1g": "gatheronly", "p1n": "nogather"}.get(phases, "full")
            _edge_layer(nc, tc, nch, ch_off, stab, dtab, maskt,
                        xs_full=xs1_full, row=ROW1, cph=C1, brep=b1r,
                        w_next=w2, ident=ident,
                        next_row=ROW2, xs_next_my=xs2_my,
                        t_all=None, wf1=None, bf1r=None, mode=emode)
            if phases in ("p1", "p1g", "p1n"):
                _dummy_out(nc, tc, out_d)
                return

            nc.gpsimd.collective_compute(
                "AllGather", mybir.AluOpType.bypass,
                replica_groups=[list(range(NCORES))],
                ins=[xs2_my.opt()], outs=[xs2_full.opt()])

            # ============ P2: layer-2 edges + head GEMM + stats ============
            with tc.tile_pool(name="hd", bufs=1) as hp:
                t_all = hp.tile([DBLK, NBLK, 128], F32)
                _edge_layer(nc, tc, nch, ch_off, stab, dtab, maskt,
                            xs_full=xs2_full, row=ROW2, cph=C2, brep=b2r,
                            w_next=None, ident=ident,
                            next_row=None, xs_next_my=None,
                            t_all=t_all, wf1=wf1, bf1r=bf1r)

                # ---- batchnorm stats + allreduce
                with tc.tile_pool(name="bns", bufs=1) as bp, \
                     tc.tile_pool(name="bnq", bufs=2) as bq, \
                     tc.tile_pool(name="bnp", bufs=1, space="PSUM") as bpp:
                    pstat_s = bpp.tile([1, 128], F32)
                    pstat_q = bpp.tile([1, 128], F32)
                    for b in range(NBLK):
                        sq = bq.tile([DBLK, 128], F32, tag="sq")
                        nc.scalar.activation(sq[:], t_all[:, b, :],
                                             mybir.ActivationFunctionType.Square)
                        nc.tensor.matmul(pstat_s[:], ones[0:DBLK, 0:1],
                                         t_all[:, b, :],
                                         start=(b == 0), stop=(b == NBLK - 1))
                        nc.tensor.matmul(pstat_q[:], ones[0:DBLK, 0:1], sq[:],
                                         start=(b == 0), stop=(b == NBLK - 1))
                    stsb = bp.tile([1, 256], F32)
                    nc.vector.tensor_copy(stsb[:, 0:128], pstat_s[:])
                    nc.vector.tensor_copy(stsb[:, 128:256], pstat_q[:])
                    nc.sync.dma_start(out=st_my[:], in_=stsb[:])
                    nc.gpsimd.collective_compute(
                        "AllReduce", mybir.AluOpType.add,
                        replica_groups=[list(range(NCORES))],
                        ins=[st_my.opt()], outs=[st_all.opt()])
                    st = bp.tile([1, 256], F32)
                    nc.sync.dma_start(out=st[:], in_=st_all[:])
                    mean = bp.tile([1, 128], F32)
                    nc.vector.tensor_scalar_mul(mean[:], st[:, 0:128], 1.0 / N)
                    msq = bp.tile([1, 128], F32)
                    nc.vector.tensor_scalar_mul(msq[:], st[:, 128:256], 1.0 / N)
                    m2 = bp.tile([1, 128], F32)
                    nc.scalar.activation(m2[:], mean[:],
                                         mybir.ActivationFunctionType.Square)
                    var = bp.tile([1, 128], F32)
                    nc.vector.tensor_sub(var[:], msq[:], m2[:])
                    nc.vector.tensor_scalar_add(var[:], var[:], 1e-5)
                    sd = bp.tile([1, 128], F32)
                    nc.scalar.activation(sd[:], var[:],
                                         mybir.ActivationFunctionType.Sqrt)
                    rsd = bp.tile([1, 128], F32)
                    nc.vector.reciprocal(rsd[:], sd[:])
                    ss = bp.tile([1, 256], F32)
                    nc.vector.tensor_mul(ss[:, 0:128], rsd[:], gam[:])
                    tmp = bp.tile([1, 128], F32)
                    nc.vector.tensor_mul(tmp[:], mean[:], ss[:, 0:128])
                    nc.vector.tensor_sub(ss[:, 128:256], bet[:], tmp[:])
                    prep = bpp.tile([128, 256], F32)
                    nc.tensor.matmul(prep[:], ones[0:1, 0:128], ss[0:1, :],
                                     start=True, stop=True)
                    srep = hp.tile([128, 256], F32)
                    nc.vector.tensor_copy(srep[:], prep[:])

                # ---- P3: finalize head per block
                with tc.tile_pool(name="f3", bufs=3) as fp, \
                     tc.tile_pool(name="f3p", bufs=2, space="PSUM") as fpp:
                    for b in range(NBLK):
                        tmpf = fp.tile([DBLK, 128], F32, tag="tmpf")
                        nc.vector.tensor_mul(tmpf[:], t_all[:, b, :],
                                             srep[0:DBLK, 0:128])
                        nc.vector.tensor_add(tmpf[:], tmpf[:],
                                             srep[0:DBLK, 128:256])
                        tn = fp.tile([DBLK, 128], BF16, tag="tn")
                        nc.vector.tensor_scalar_max(tn[:], tmpf[:], 0.0)
                        ptn = fpp.tile([128, DBLK], BF16, tag="ptn")
                        nc.tensor.transpose(ptn[:], tn[:], ident[0:DBLK, 0:DBLK])
                        tT = fp.tile([128, DBLK], BF16, tag="tT")
                        nc.vector.tensor_copy(tT[:], ptn[:])
                        po = fpp.tile([DBLK, OUT], F32, tag="po")
                        nc.tensor.matmul(po[:], tT[:], wf2[:], start=True, stop=True)
                        ot = fp.tile([DBLK, OUT], F32, tag="ot")
                        nc.vector.tensor_add(ot[:], po[:], bf2r[0:DBLK, :])
                        ex = fp.tile([DBLK, OUT], F32, tag="ex")
                        acc = fp.tile([DBLK, 1], F32, tag="acc")
                        nc.scalar.activation(ex[:], ot[:],
                                             mybir.ActivationFunctionType.Exp,
                                             accum_out=acc[:])
                        ln = fp.tile([DBLK, 1], F32, tag="ln")
                        nc.scalar.activation(ln[:], acc[:],
                                             mybir.ActivationFunctionType.Ln)
                        res = fp.tile([DBLK, OUT], F32, tag="res")
                        nc.vector.tensor_scalar_sub(res[:], ot[:], ln[:])
                        nc.sync.dma_start(
                            out=out_d[b * DBLK:(b + 1) * DBLK, :], in_=res[:])

    with tile.TileContext(nc) as tc:
        body(tc)
    nc.compile()
    return nc


def _dummy_out(nc, tc, out_d):
    with tc.tile_pool(name="dz", bufs=1) as zp:
        z = zp.tile([DBLK, OUT], F32)
        nc.vector.memset(z[:], 0.0)
        for b in range(NBLK):
            nc.sync.dma_start(out=out_d[b * DBLK:(b + 1) * DBLK, :], in_=z[:])


def _edge_layer(nc, tc, nch, ch_off, stab, dtab, maskt, *, xs_full, row, cph,
                brep, w_next, ident, next_row, xs_next_my,
                t_all, wf1, bf1r, mode="full"):
    """Edge phase for one GAT layer + the following per-block GEMM.

    Layer 1 (w_next given): produces xs_next rows (layer-2 aug features).
    Layer 2 (t_all given): produces t = h2 @ wf1 + bf1 rows + BN stat sums.
    """
    acol = 512 if row == ROW1 else 1024     # bf16 col where fp32 a-vals live
    nheads_cols = H * cph                   # 512 or 1024
    with tc.tile_pool(name=f"e{row}s", bufs=2) as ep, \
         tc.tile_pool(name=f"e{row}m", bufs=4) as mp, \
         tc.tile_pool(name=f"e{row}p", bufs=2, space="PSUM") as pp, \
         tc.tile_pool(name=f"e{row}d", bufs=1, space="PSUM") as dp, \
         tc.tile_pool(name=f"e{row}pt", bufs=1, space="PSUM") as tp:
        nchmax = max(nch)
        for b in range(len(nch)):
            nb = nch[b]
            k0 = int(ch_off[b])
            ni = nb * 128
            tslice = slice(k0 * 8, (k0 + nb) * 8)
            # ---- gather a-rows (src and dst), compute p = exp(leaky(e))
            def gather(out_tile, src_cols, tab, elem):
                # SWDGE gathers fail above ~1024 idxs; split into <=1024 pieces
                for g0 in range(0, nb, GMAX):
                    ng = min(GMAX, nb - g0)
                    nc.gpsimd.dma_gather(
                        out_tile[:, g0:g0 + ng, :],
                        xs_full[:, src_cols:src_cols + elem],
                        tab[:, (k0 + g0) * 8:(k0 + g0 + ng) * 8],
                        num_idxs=ng * 128, num_idxs_reg=ng * 128,
                        elem_size=elem, elem_step=row)

            gas = ep.tile([128, nchmax, 128], BF16, tag="gas")
            gad = ep.tile([128, nchmax, 128], BF16, tag="gad")
            if mode == "nogather":
                nc.vector.memset(gas[:, 0:nb, :].bitcast(F32), 0.01)
                nc.vector.memset(gad[:, 0:nb, :].bitcast(F32), 0.01)
            else:
                gather(gas, acol, stab, 128)
                gather(gad, acol, dtab, 128)
            gasf = gas[:].bitcast(F32)      # [128, nchmax, 64]
            gadf = gad[:].bitcast(F32)
            et = ep.tile([128, nchmax, 2], F32, tag="et")
            nc.vector.tensor_add(et[:, 0:nb, :], gasf[:, 0:nb, 0:2],
                                 gadf[:, 0:nb, 2:4])
            e2 = ep.tile([128, nchmax, 2], F32, tag="e2")
            nc.vector.tensor_scalar_mul(e2[:, 0:nb, :], et[:, 0:nb, :], 0.2)
            nc.vector.tensor_max(et[:, 0:nb, :], et[:, 0:nb, :], e2[:, 0:nb, :])
            pb = ep.tile([128, nchmax, 2], F32, tag="pb")
            nc.scalar.activation(pb[:, 0:nb, :], et[:, 0:nb, :],
                                 mybir.ActivationFunctionType.Exp)
            pbb = ep.tile([128, nchmax, 2], BF16, tag="pbb")
            nc.vector.tensor_copy(pbb[:, 0:nb, :], pb[:, 0:nb, :])
            # ---- gather msg rows per head
            gms = []
            for h in range(H):
                gm = ep.tile([128, nchmax, cph], BF16, tag=f"gm{h}")
                if mode == "nogather":
                    nc.vector.memset(gm[:, 0:nb, :], 0.01)
                else:
                    gather(gm, h * cph, stab, cph)
                gms.append(gm)
            if mode == "gatheronly":
                continue
            # ---- chunked mask matmuls (one accumulation chain per PSUM bank)
            agg_a = pp.tile([DBLK, cph], F32, tag="agg0")
            agg_b = pp.tile([DBLK, cph], F32, tag="agg1")
            aggs = [agg_a, agg_b]
            den = dp.tile([DBLK, 2], F32, tag="den")
            for k in range(nb):
                mk = maskt[:, (k0 + k) * DBLK:(k0 + k + 1) * DBLK]
                for h in range(H):
                    sm = mp.tile([128, DBLK], BF16, tag="sm")
                    nc.scalar.activation(sm[:], mk,
                                         mybir.ActivationFunctionType.Copy,
                                         scale=pb[:, k, h:h + 1])
                    nc.tensor.matmul(aggs[h][:], sm[:], gms[h][:, k, :],
                                     start=(k == 0), stop=(k == nb - 1))
                nc.tensor.matmul(den[:], mk, pbb[:, k, :],
                                 start=(k == 0), stop=(k == nb - 1))
            # ---- normalize + bias + relu -> h rows (bf16)
            rec = ep.tile([DBLK, 2], F32, tag="rec")
            nc.vector.reciprocal(rec[:], den[:])
            hs = ep.tile([DBLK, nheads_cols], BF16, tag="hs")
            for h in range(H):
                tmp = ep.tile([DBLK, cph], F32, tag="tmpn")
                nc.vector.tensor_scalar_mul(tmp[:], aggs[h][:], rec[:, h:h + 1])
                nc.vector.tensor_add(tmp[:], tmp[:],
                                     brep[0:DBLK, h * cph:(h + 1) * cph])
                nc.vector.tensor_scalar_max(hs[:, h * cph:(h + 1) * cph],
                                            tmp[:], 0.0)
            # ---- transpose h rows for the next GEMM
            nkc = nheads_cols // 128
            hT = ep.tile([128, nkc, DBLK], BF16, tag="hT")
            for kc in range(nkc):
                pt = tp.tile([128, DBLK], BF16, tag="pt")
                nc.tensor.transpose(pt[:], hs[:, kc * 128:(kc + 1) * 128],
                                    ident[0:DBLK, 0:DBLK])
                nc.vector.tensor_copy(hT[:, kc, :], pt[:])
            if w_next is not None:
                # layer-2 aug GEMM: [125, 1028] in 4 quarters of 257
                stg = ep.tile([DBLK, next_row], BF16, tag="stg2")
                nc.vector.memset(stg[:, 1032:next_row], 0.0)
                for q in range(4):
                    p2 = tp.tile([DBLK, 257], F32, tag="p2")
                    for kc in range(4):
                        nc.tensor.matmul(p2[:], hT[:, kc, :],
                                         w_next[:, kc, q * 257:(q + 1) * 257],
                                         start=(kc == 0), stop=(kc == 3))
                    if q < 3:
                        nc.vector.tensor_copy(stg[:, q * 257:(q + 1) * 257], p2[:])
                    else:
                        nc.vector.tensor_copy(stg[:, 771:1024], p2[:, 0:253])
                        stgf = stg[:].bitcast(F32)
                        nc.vector.tensor_copy(stgf[:, 512:516], p2[:, 253:257])
                nc.sync.dma_start(out=xs_next_my[b * DBLK:(b + 1) * DBLK, :],
                                  in_=stg[:])
            else:
                # head GEMM: t = h2 @ wf1 + bf1; accumulate BN stat sums
                pw = tp.tile([DBLK, 128], F32, tag="pw")
                for kc in range(8):
                    nc.tensor.matmul(pw[:], hT[:, kc, :], wf1[:, kc, :],
                                     start=(kc == 0), stop=(kc == 7))
                nc.vector.tensor_add(t_all[:, b, :], pw[:], bf1r[0:DBLK, :])


# ------------------------------------------------------------------ runner

def _make_in_maps(inputs, src_tabs, dst_tabs, masks):
    x = np.asarray(inputs["x"], np.float32)
    w1s = np.asarray(inputs["w1s"], np.float32)
    w1d = np.asarray(inputs["w1d"], np.float32)
    w2s = np.asarray(inputs["w2s"], np.float32)
    w2d = np.asarray(inputs["w2d"], np.float32)
    w1aug = np.concatenate([w1s, _fold_att(w1s, np.asarray(inputs["a1s"]), C1),
                            _fold_att(w1d, np.asarray(inputs["a1d"]), C1)], 1)
    w2aug = np.concatenate([w2s, _fold_att(w2s, np.asarray(inputs["a2s"]), C2),
                            _fold_att(w2d, np.asarray(inputs["a2d"]), C2)], 1)

    common = {
        "w1aug": bf(w1aug),
        "w2aug": bf(w2aug),
        "wf1": bf(np.asarray(inputs["wf1"])),
        "wf2": bf(np.asarray(inputs["wf2"])),
        "b1rep": np.tile(np.asarray(inputs["b1"], np.float32)[None, :], (128, 1)),
        "b2rep": np.tile(np.asarray(inputs["b2"], np.float32)[None, :], (128, 1)),
        "bf1rep": np.tile(np.asarray(inputs["bf1"], np.float32)[None, :], (128, 1)),
        "bf2rep": np.tile(np.asarray(inputs["bf2"], np.float32)[None, :], (128, 1)),
        "gb": np.stack([np.asarray(inputs["gamma"], np.float32),
                        np.asarray(inputs["beta"], np.float32)]),
        "ident": bf(np.eye(128, dtype=np.float32)),
        "ones": np.ones((128, 128), np.float32),
    }
    in_maps = []
    for c in range(NCORES):
        m = dict(common)
        m["xT"] = np.ascontiguousarray(bf(x[c * NLOC:(c + 1) * NLOC].T))
        m["src_tab"] = src_tabs[c]
        m["dst_tab"] = dst_tabs[c]
        m["mask"] = masks[c]
        in_maps.append(m)
    return in_maps


def kernel(**inputs):
    ei = np.asarray(inputs["edge_index"])
    nch, ch_off, epad, src_tabs, dst_tabs, masks = _prep_graph(ei)
    in_maps = _make_in_maps(inputs, src_tabs, dst_tabs, masks)
    nc = build(nch, ch_off)
    res = run_bass_kernel_spmd(nc, in_maps, list(range(NCORES)))
    out = np.concatenate([res.results[c]["out"] for c in range(NCORES)], 0)
    return out.astype(np.float32)


if __name__ == "__main__":
    ei = np.random.default_rng(0).integers(0, N, (2, E))
    print("prep ok:", _prep_graph(ei)[0])


# revision 43
# speedup vs baseline: 67.8550x; 67.8550x over previous
"""GAT (2-layer, 2-head) + MLP head on 8 Trainium2 NeuronCores.

Strategy (graph/data parallel, per sharding hint):
  - Nodes sharded 1250/core; edges partitioned by dst and sorted by dst on
    the host, grouped into dst-blocks of 125 rows, edge chunks of 128.
  - Attention logits fold into extra GEMM columns on the host:
    a_s = x @ (W_s . att_s), a_d = x @ (W_d . att_d), so xd is never built.
  - Per-layer per-core: local GEMM -> AllGather of [N, row] bf16 feature
    rows (msg cols + fp32 a-cols packed in-row) -> dma_gather per edge of
    a-rows (src & dst) and msg rows (per head) -> p = exp(leaky(a_s+a_d))
    -> segment aggregation as chunked mask matmuls on the PE
    (mask scaled per-edge by p via ACT; denominator via raw-mask matmul)
    -> normalize by 1/denom, bias, relu.
  - Head: t = h2 @ wf1 + bf1 per block, batchnorm stats via ones-matmul +
    AllReduce, normalize + relu + wf2 + log_softmax.

kernel(**inputs) takes FULL inputs, returns FULL [10000, 3] fp32 output.
"""
import sys

sys.path.insert(0, "/opt/trn_rl_repo")

import numpy as np
import ml_dtypes

import concourse.bass as bass
import concourse.bacc as bacc
import concourse.mybir as mybir
import concourse.tile as tile
from concourse.bass_utils import run_bass_kernel_spmd

F32 = mybir.dt.float32
BF16 = mybir.dt.bfloat16
I16 = mybir.dt.int16

N = 10000
E = 160000
H = 2
D_IN = 256
C1 = 256          # per-head channels conv1 -> msg row 512
C2 = 512          # per-head channels conv2 -> msg row 1024
OUT = 3
NCORES = 8
NLOC = N // NCORES          # 1250
DBLK = 125                  # dst rows per block
NBLK = NLOC // DBLK         # 10

ROW1 = 640                  # bf16 cols: 512 msg | 4xf32 a (cols 512..519) | pad
ROW2 = 1152                 # bf16 cols: 1024 msg | 4xf32 a (1024..1031) | pad
GMAX = 8                    # max 128-chunks per dma_gather (1024 idxs)

bf = lambda a: np.asarray(a, ml_dtypes.bfloat16)


# ----------------------------------------------------------------- host prep

def _wrap_tab(idx, epad):
    """dma_gather index table: [128, epad/16] int16, token s at [s%16, s//16],
    replicated across the 8 groups of 16 partitions."""
    t = idx.astype(np.int16).reshape(epad // 16, 16).T        # [16, epad/16]
    return np.ascontiguousarray(np.tile(t, (8, 1)))


def _prep_graph(edge_index):
    src = np.concatenate([np.asarray(edge_index[0]), np.arange(N)]).astype(np.int64)
    dst = np.concatenate([np.asarray(edge_index[1]), np.arange(N)]).astype(np.int64)
    order = np.argsort(dst, kind="stable")
    src, dst = src[order], dst[order]
    core = dst // NLOC
    blk = (dst % NLOC) // DBLK
    counts = np.zeros((NCORES, NBLK), np.int64)
    np.add.at(counts, (core, blk), 1)
    nch = [max(1, int(-(-counts[:, b].max() // 128))) for b in range(NBLK)]
    totch = sum(nch)
    epad = totch * 128
    ch_off = np.concatenate([[0], np.cumsum(nch)]).astype(np.int64)

    src_tabs, dst_tabs, masks = [], [], []
    for c in range(NCORES):
        s_pad = np.zeros(epad, np.int64)
        d_pad = np.zeros(epad, np.int64)
        mask = np.zeros((128, totch * DBLK), np.float32)
        for b in range(NBLK):
            sel = (core == c) & (blk == b)
            ss, dd = src[sel], dst[sel]
            cnt = ss.shape[0]
            base = int(ch_off[b]) * 128
            s_pad[base:base + cnt] = ss
            d_pad[base:base + cnt] = dd
            j = np.arange(cnt)
            kk = int(ch_off[b]) + j // 128
            pp = j % 128
            dloc = (dd % NLOC) % DBLK
            mask[pp, kk * DBLK + dloc] = 1.0
        src_tabs.append(_wrap_tab(s_pad, epad))
        dst_tabs.append(_wrap_tab(d_pad, epad))
        masks.append(bf(mask))
    return nch, ch_off, epad, src_tabs, dst_tabs, masks


def _fold_att(W, att, C):
    # v[k, h] = sum_c W[k, h*C + c] * att[h, c]
    K = W.shape[0]
    return np.stack([W[:, h * C:(h + 1) * C] @ att[h] for h in range(H)], 1)


# ------------------------------------------------------------- kernel build

def build(nch, ch_off, phases="full", repeat=1):
    totch = sum(nch)
    epad = totch * 128
    nc = bacc.Bacc("TRN2", target_bir_lowering=False, debug=False,
                   num_devices=NCORES, num_swdge_queues=2)

    def din(name, shape, dt):
        return nc.dram_tensor(name, shape, dt, kind="ExternalInput")

    xT_d = din("xT", [D_IN, NLOC], BF16)
    w1_d = din("w1aug", [D_IN, 516], BF16)
    w2_d = din("w2aug", [4 * 128, 1028], BF16)
    wf1_d = din("wf1", [8 * 128, 128], BF16)
    wf2_d = din("wf2", [128, OUT], BF16)
    b1r_d = din("b1rep", [128, 512], F32)
    b2r_d = din("b2rep", [128, 1024], F32)
    bf1r_d = din("bf1rep", [128, 128], F32)
    bf2r_d = din("bf2rep", [128, OUT], F32)
    gb_d = din("gb", [2, 128], F32)          # gamma, beta
    ident_d = din("ident", [128, 128], BF16)
    ones_d = din("ones", [128, 128], F32)
    stab_d = din("src_tab", [128, epad // 16], I16)
    dtab_d = din("dst_tab", [128, epad // 16], I16)
    mask_d = din("mask", [128, totch * DBLK], BF16)
    out_d = nc.dram_tensor("out", [NLOC, OUT], F32, kind="ExternalOutput")

    def body(tc):
        with tc.tile_pool(name="const", bufs=1) as cp, \
             tc.tile_pool(name="dram", bufs=1, space="DRAM") as dram:
            # ---- resident SBUF constants
            xT = cp.tile([128, 2, NLOC], BF16)
            for kc in range(2):
                nc.sync.dma_start(out=xT[:, kc, :], in_=xT_d[kc * 128:(kc + 1) * 128, :])
            w1 = cp.tile([128, 2, 516], BF16)
            for kc in range(2):
                nc.sync.dma_start(out=w1[:, kc, :], in_=w1_d[kc * 128:(kc + 1) * 128, :])
            w2 = cp.tile([128, 4, 1028], BF16)
            for kc in range(4):
                nc.sync.dma_start(out=w2[:, kc, :], in_=w2_d[kc * 128:(kc + 1) * 128, :])
            wf1 = cp.tile([128, 8, 128], BF16)
            for kc in range(8):
                nc.sync.dma_start(out=wf1[:, kc, :], in_=wf1_d[kc * 128:(kc + 1) * 128, :])
            wf2 = cp.tile([128, OUT], BF16)
            nc.sync.dma_start(out=wf2[:], in_=wf2_d[:])
            b1r = cp.tile([128, 512], F32)
            nc.sync.dma_start(out=b1r[:], in_=b1r_d[:])
            b2r = cp.tile([128, 1024], F32)
            nc.sync.dma_start(out=b2r[:], in_=b2r_d[:])
            bf1r = cp.tile([128, 128], F32)
            nc.sync.dma_start(out=bf1r[:], in_=bf1r_d[:])
            bf2r = cp.tile([128, OUT], F32)
            nc.sync.dma_start(out=bf2r[:], in_=bf2r_d[:])
            gam = cp.tile([1, 128], F32)
            nc.sync.dma_start(out=gam[:], in_=gb_d[0:1, :])
            bet = cp.tile([1, 128], F32)
            nc.sync.dma_start(out=bet[:], in_=gb_d[1:2, :])
            ident = cp.tile([128, 128], BF16)
            nc.sync.dma_start(out=ident[:], in_=ident_d[:])
            ones = cp.tile([128, 128], F32)
            nc.sync.dma_start(out=ones[:], in_=ones_d[:])
            stab = cp.tile([128, epad // 16], I16)
            nc.sync.dma_start(out=stab[:], in_=stab_d[:])
            dtab = cp.tile([128, epad // 16], I16)
            nc.sync.dma_start(out=dtab[:], in_=dtab_d[:])
            maskt = cp.tile([128, totch * DBLK], BF16)
            nc.sync.dma_start(out=maskt[:], in_=mask_d[:])

            xs1_my = dram.tile([NLOC, ROW1], BF16)
            xs1_full = dram.tile([N, ROW1], BF16)
            xs2_my = dram.tile([NLOC, ROW2], BF16)
            xs2_full = dram.tile([N, ROW2], BF16)
            st_my = dram.tile([1, 256], F32)
            st_all = dram.tile([1, 256], F32)

            # ================= P0: layer-1 local GEMM =================
            for _rep in range(repeat):
              with tc.tile_pool(name="p0s", bufs=3) as sp, \
                 tc.tile_pool(name="p0p", bufs=2, space="PSUM") as pp:
                for b in range(NBLK):
                    ph0 = pp.tile([DBLK, 258], F32, tag="ph0")
                    ph1 = pp.tile([DBLK, 258], F32, tag="ph1")
                    for kc in range(2):
                        lhs = xT[:, kc, b * DBLK:(b + 1) * DBLK]
                        nc.tensor.matmul(ph0[:], lhs, w1[:, kc, 0:258],
                                         start=(kc == 0), stop=(kc == 1))
                        nc.tensor.matmul(ph1[:], lhs, w1[:, kc, 258:516],
                                         start=(kc == 0), stop=(kc == 1))
                    stg = sp.tile([DBLK, ROW1], BF16, tag="stg")
                    nc.vector.memset(stg[:, 520:ROW1], 0.0)
                    nc.vector.tensor_copy(stg[:, 0:258], ph0[:])
                    nc.vector.tensor_copy(stg[:, 258:512], ph1[:, 0:254])
                    stgf = stg[:].bitcast(F32)          # [DBLK, 320]
                    nc.vector.tensor_copy(stgf[:, 256:260], ph1[:, 254:258])
                    nc.sync.dma_start(out=xs1_my[b * DBLK:(b + 1) * DBLK, :],
                                      in_=stg[:])

            if phases == "p0":
                _dummy_out(nc, tc, out_d)
                return
            nc.gpsimd.collective_compute(
                "AllGather", mybir.AluOpType.bypass,
                replica_groups=[list(range(NCORES))],
                ins=[xs1_my.opt()], outs=[xs1_full.opt()])
            if phases == "ag1":
                _dummy_out(nc, tc, out_d)
                return

            # ============ P1: layer-1 edges + layer-2 input GEMM ============
            emode = {"p1g": "gatheronly", "p1n": "nogather"}.get(phases, "full")
            _edge_layer(nc, tc, nch, ch_off, stab, dtab, maskt,
                        xs_full=xs1_full, row=ROW1, cph=C1, brep=b1r,
                        w_next=w2, ident=ident,
                        next_row=ROW2, xs_next_my=xs2_my,
                        t_all=None, wf1=None, bf1r=None, mode=emode)
            if phases in ("p1", "p1g", "p1n"):
                _dummy_out(nc, tc, out_d)
                return

            nc.gpsimd.collective_compute(
                "AllGather", mybir.AluOpType.bypass,
                replica_groups=[list(range(NCORES))],
                ins=[xs2_my.opt()], outs=[xs2_full.opt()])

            # ============ P2: layer-2 edges + head GEMM + stats ============
            with tc.tile_pool(name="hd", bufs=1) as hp:
                t_all = hp.tile([DBLK, NBLK, 128], F32)
                _edge_layer(nc, tc, nch, ch_off, stab, dtab, maskt,
                            xs_full=xs2_full, row=ROW2, cph=C2, brep=b2r,
                            w_next=None, ident=ident,
                            next_row=None, xs_next_my=None,
                            t_all=t_all, wf1=wf1, bf1r=bf1r)

                # ---- batchnorm stats + allreduce
                with tc.tile_pool(name="bns", bufs=1) as bp, \
                     tc.tile_pool(name="bnq", bufs=2) as bq, \
                     tc.tile_pool(name="bnp", bufs=1, space="PSUM") as bpp:
                    pstat_s = bpp.tile([1, 128], F32)
                    pstat_q = bpp.tile([1, 128], F32)
                    for b in range(NBLK):
                        sq = bq.tile([DBLK, 128], F32, tag="sq")
                        nc.scalar.activation(sq[:], t_all[:, b, :],
                                             mybir.ActivationFunctionType.Square)
                        nc.tensor.matmul(pstat_s[:], ones[0:DBLK, 0:1],
                                         t_all[:, b, :],
                                         start=(b == 0), stop=(b == NBLK - 1))
                        nc.tensor.matmul(pstat_q[:], ones[0:DBLK, 0:1], sq[:],
                                         start=(b == 0), stop=(b == NBLK - 1))
                    stsb = bp.tile([1, 256], F32)
                    nc.vector.tensor_copy(stsb[:, 0:128], pstat_s[:])
                    nc.vector.tensor_copy(stsb[:, 128:256], pstat_q[:])
                    nc.sync.dma_start(out=st_my[:], in_=stsb[:])
                    nc.gpsimd.collective_compute(
                        "AllReduce", mybir.AluOpType.add,
                        replica_groups=[list(range(NCORES))],
                        ins=[st_my.opt()], outs=[st_all.opt()])
                    st = bp.tile([1, 256], F32)
                    nc.sync.dma_start(out=st[:], in_=st_all[:])
                    mean = bp.tile([1, 128], F32)
                    nc.vector.tensor_scalar_mul(mean[:], st[:, 0:128], 1.0 / N)
                    msq = bp.tile([1, 128], F32)
                    nc.vector.tensor_scalar_mul(msq[:], st[:, 128:256], 1.0 / N)
                    m2 = bp.tile([1, 128], F32)
                    nc.scalar.activation(m2[:], mean[:],
                                         mybir.ActivationFunctionType.Square)
                    var = bp.tile([1, 128], F32)
                    nc.vector.tensor_sub(var[:], msq[:], m2[:])
                    nc.vector.tensor_scalar_add(var[:], var[:], 1e-5)
                    sd = bp.tile([1, 128], F32)
                    nc.scalar.activation(sd[:], var[:],
                                         mybir.ActivationFunctionType.Sqrt)
                    rsd = bp.tile([1, 128], F32)
                    nc.vector.reciprocal(rsd[:], sd[:])
                    ss = bp.tile([1, 256], F32)
                    nc.vector.tensor_mul(ss[:, 0:128], rsd[:], gam[:])
                    tmp = bp.tile([1, 128], F32)
                    nc.vector.tensor_mul(tmp[:], mean[:], ss[:, 0:128])
                    nc.vector.tensor_sub(ss[:, 128:256], bet[:], tmp[:])
                    prep = bpp.tile([128, 256], F32)
                    nc.tensor.matmul(prep[:], ones[0:1, 0:128], ss[0:1, :],
                                     start=True, stop=True)
                    srep = hp.tile([128, 256], F32)
                    nc.vector.tensor_copy(srep[:], prep[:])

                # ---- P3: finalize head per block
                with tc.tile_pool(name="f3", bufs=3) as fp, \
                     tc.tile_pool(name="f3p", bufs=2, space="PSUM") as fpp:
                    for b in range(NBLK):
                        tmpf = fp.tile([DBLK, 128], F32, tag="tmpf")
                        nc.vector.tensor_mul(tmpf[:], t_all[:, b, :],
                                             srep[0:DBLK, 0:128])
                        nc.vector.tensor_add(tmpf[:], tmpf[:],
                                             srep[0:DBLK, 128:256])
                        tn = fp.tile([DBLK, 128], BF16, tag="tn")
                        nc.vector.tensor_scalar_max(tn[:], tmpf[:], 0.0)
                        ptn = fpp.tile([128, DBLK], BF16, tag="ptn")
                        nc.tensor.transpose(ptn[:], tn[:], ident[0:DBLK, 0:DBLK])
                        tT = fp.tile([128, DBLK], BF16, tag="tT")
                        nc.vector.tensor_copy(tT[:], ptn[:])
                        po = fpp.tile([DBLK, OUT], F32, tag="po")
                        nc.tensor.matmul(po[:], tT[:], wf2[:], start=True, stop=True)
                        ot = fp.tile([DBLK, OUT], F32, tag="ot")
                        nc.vector.tensor_add(ot[:], po[:], bf2r[0:DBLK, :])
                        ex = fp.tile([DBLK, OUT], F32, tag="ex")
                        acc = fp.tile([DBLK, 1], F32, tag="acc")
                        nc.scalar.activation(ex[:], ot[:],
                                             mybir.ActivationFunctionType.Exp,
                                             accum_out=acc[:])
                        ln = fp.tile([DBLK, 1], F32, tag="ln")
                        nc.scalar.activation(ln[:], acc[:],
                                             mybir.ActivationFunctionType.Ln)
                        res = fp.tile([DBLK, OUT], F32, tag="res")
                        nc.vector.tensor_scalar_sub(res[:], ot[:], ln[:])
                        nc.sync.dma_start(
                            out=out_d[b * DBLK:(b + 1) * DBLK, :], in_=res[:])

    with tile.TileContext(nc) as tc:
        body(tc)
    nc.compile()
    return nc


def _dummy_out(nc, tc, out_d):
    with tc.tile_pool(name="dz", bufs=1) as zp:
        z = zp.tile([DBLK, OUT], F32)
        nc.vector.memset(z[:], 0.0)
        for b in range(NBLK):
            nc.sync.dma_start(out=out_d[b * DBLK:(b + 1) * DBLK, :], in_=z[:])


def _edge_layer(nc, tc, nch, ch_off, stab, dtab, maskt, *, xs_full, row, cph,
                brep, w_next, ident, next_row, xs_next_my,
                t_all, wf1, bf1r, mode="full"):
    """Edge phase for one GAT layer + the following per-block GEMM.

    Layer 1 (w_next given): produces xs_next rows (layer-2 aug features).
    Layer 2 (t_all given): produces t = h2 @ wf1 + bf1 rows + BN stat sums.
    """
    acol = 512 if row == ROW1 else 1024     # bf16 col where fp32 a-vals live
    nheads_cols = H * cph                   # 512 or 1024
    with tc.tile_pool(name=f"e{row}s", bufs=2) as ep, \
         tc.tile_pool(name=f"e{row}m", bufs=6) as mp, \
         tc.tile_pool(name=f"e{row}p", bufs=2, space="PSUM") as pp, \
         tc.tile_pool(name=f"e{row}d", bufs=1, space="PSUM") as dp, \
         tc.tile_pool(name=f"e{row}pt", bufs=2, space="PSUM") as tp, \
         tc.tile_pool(name=f"e{row}pq", bufs=1, space="PSUM") as tq:
        nchmax = max(nch)
        for b in range(len(nch)):
            nb = nch[b]
            k0 = int(ch_off[b])
            ni = nb * 128
            tslice = slice(k0 * 8, (k0 + nb) * 8)
            # ---- gather a-rows (src and dst), compute p = exp(leaky(e))
            def gather(out_tile, src_cols, tab, elem, q=0):
                # SWDGE gathers fail above ~1024 idxs; split into <=1024 pieces
                for g0 in range(0, nb, GMAX):
                    ng = min(GMAX, nb - g0)
                    nc.gpsimd.dma_gather(
                        out_tile[:, g0:g0 + ng, :],
                        xs_full[:, src_cols:src_cols + elem],
                        tab[:, (k0 + g0) * 8:(k0 + g0 + ng) * 8],
                        num_idxs=ng * 128, num_idxs_reg=ng * 128,
                        elem_size=elem, elem_step=row, queue_num=q)

            # one full-row gather by src: messages for both heads + a_s cols
            gsrc = ep.tile([128, nchmax, row], BF16, tag="gsrc")
            gad = ep.tile([128, nchmax, 128], BF16, tag="gad")
            if mode == "nogather":
                nc.vector.memset(gsrc[:, 0:nb, :].bitcast(F32), 0.01)
                nc.vector.memset(gad[:, 0:nb, :].bitcast(F32), 0.01)
            else:
                gather(gsrc, 0, stab, row)
                gather(gad, acol, dtab, 128, q=1)
            gsrcf = gsrc[:].bitcast(F32)    # [128, nchmax, row//2]
            gadf = gad[:].bitcast(F32)
            et = ep.tile([128, nchmax, 2], F32, tag="et")
            nc.vector.tensor_add(et[:, 0:nb, :],
                                 gsrcf[:, 0:nb, acol // 2:acol // 2 + 2],
                                 gadf[:, 0:nb, 2:4])
            e2 = ep.tile([128, nchmax, 2], F32, tag="e2")
            nc.vector.tensor_scalar_mul(e2[:, 0:nb, :], et[:, 0:nb, :], 0.2)
            nc.vector.tensor_max(et[:, 0:nb, :], et[:, 0:nb, :], e2[:, 0:nb, :])
            pb = ep.tile([128, nchmax, 2], F32, tag="pb")
            nc.scalar.activation(pb[:, 0:nb, :], et[:, 0:nb, :],
                                 mybir.ActivationFunctionType.Exp)
            pbb = ep.tile([128, nchmax, 2], BF16, tag="pbb")
            nc.vector.tensor_copy(pbb[:, 0:nb, :], pb[:, 0:nb, :])
            if mode == "gatheronly":
                continue
            # ---- chunked mask matmuls (one accumulation chain per PSUM bank)
            agg_a = pp.tile([DBLK, cph], F32, tag="agg0")
            agg_b = pp.tile([DBLK, cph], F32, tag="agg1")
            aggs = [agg_a, agg_b]
            den = dp.tile([DBLK, 2], F32, tag="den")
            for k in range(nb):
                mk = maskt[:, (k0 + k) * DBLK:(k0 + k + 1) * DBLK]
                for h in range(H):
                    sm = mp.tile([128, DBLK], BF16, tag=f"sm{h}")
                    if h == 0:
                        # head 0 on DVE, head 1 on ACT: balance engines
                        nc.vector.tensor_scalar_mul(sm[:], mk, pb[:, k, 0:1])
                    else:
                        nc.scalar.activation(sm[:], mk,
                                             mybir.ActivationFunctionType.Copy,
                                             scale=pb[:, k, h:h + 1])
                    nc.tensor.matmul(aggs[h][:], sm[:],
                                     gsrc[:, k, h * cph:(h + 1) * cph],
                                     start=(k == 0), stop=(k == nb - 1))
                nc.tensor.matmul(den[:], mk, pbb[:, k, :],
                                 start=(k == 0), stop=(k == nb - 1))
            # ---- normalize + bias + relu -> h rows (bf16)
            rec = ep.tile([DBLK, 2], F32, tag="rec")
            nc.vector.reciprocal(rec[:], den[:])
            hs = ep.tile([DBLK, nheads_cols], BF16, tag="hs")
            for h in range(H):
                tmp = ep.tile([DBLK, cph], F32, tag="tmpn")
                nc.vector.tensor_scalar_mul(tmp[:], aggs[h][:], rec[:, h:h + 1])
                nc.vector.tensor_add(tmp[:], tmp[:],
                                     brep[0:DBLK, h * cph:(h + 1) * cph])
                nc.vector.tensor_scalar_max(hs[:, h * cph:(h + 1) * cph],
                                            tmp[:], 0.0)
            # ---- transpose h rows for the next GEMM
            nkc = nheads_cols // 128
            hT = ep.tile([128, nkc, DBLK], BF16, tag="hT")
            for kc in range(nkc):
                pt = tp.tile([128, DBLK], BF16, tag="pt")
                nc.tensor.transpose(pt[:], hs[:, kc * 128:(kc + 1) * 128],
                                    ident[0:DBLK, 0:DBLK])
                nc.vector.tensor_copy(hT[:, kc, :], pt[:])
            if w_next is not None:
                # layer-2 aug GEMM: [125, 1028] in 4 quarters of 257
                stg = ep.tile([DBLK, next_row], BF16, tag="stg2")
                nc.vector.memset(stg[:, 1032:next_row], 0.0)
                for q in range(4):
                    p2 = tq.tile([DBLK, 257], F32, tag="p2")
                    for kc in range(4):
                        nc.tensor.matmul(p2[:], hT[:, kc, :],
                                         w_next[:, kc, q * 257:(q + 1) * 257],
                                         start=(kc == 0), stop=(kc == 3))
                    if q < 3:
                        nc.vector.tensor_copy(stg[:, q * 257:(q + 1) * 257], p2[:])
                    else:
                        nc.vector.tensor_copy(stg[:, 771:1024], p2[:, 0:253])
                        stgf = stg[:].bitcast(F32)
                        nc.vector.tensor_copy(stgf[:, 512:516], p2[:, 253:257])
                nc.sync.dma_start(out=xs_next_my[b * DBLK:(b + 1) * DBLK, :],
                                  in_=stg[:])
            else:
                # head GEMM: t = h2 @ wf1 + bf1; accumulate BN stat sums
                pw = tq.tile([DBLK, 128], F32, tag="pw")
                for kc in range(8):
                    nc.tensor.matmul(pw[:], hT[:, kc, :], wf1[:, kc, :],
                                     start=(kc == 0), stop=(kc == 7))
                nc.vector.tensor_add(t_all[:, b, :], pw[:], bf1r[0:DBLK, :])


# ------------------------------------------------------------------ runner

def _make_in_maps(inputs, src_tabs, dst_tabs, masks):
    x = np.asarray(inputs["x"], np.float32)
    w1s = np.asarray(inputs["w1s"], np.float32)
    w1d = np.asarray(inputs["w1d"], np.float32)
    w2s = np.asarray(inputs["w2s"], np.float32)
    w2d = np.asarray(inputs["w2d"], np.float32)
    w1aug = np.concatenate([w1s, _fold_att(w1s, np.asarray(inputs["a1s"]), C1),
                            _fold_att(w1d, np.asarray(inputs["a1d"]), C1)], 1)
    w2aug = np.concatenate([w2s, _fold_att(w2s, np.asarray(inputs["a2s"]), C2),
                            _fold_att(w2d, np.asarray(inputs["a2d"]), C2)], 1)

    common = {
        "w1aug": bf(w1aug),
        "w2aug": bf(w2aug),
        "wf1": bf(np.asarray(inputs["wf1"])),
        "wf2": bf(np.asarray(inputs["wf2"])),
        "b1rep": np.tile(np.asarray(inputs["b1"], np.float32)[None, :], (128, 1)),
        "b2rep": np.tile(np.asarray(inputs["b2"], np.float32)[None, :], (128, 1)),
        "bf1rep": np.tile(np.asarray(inputs["bf1"], np.float32)[None, :], (128, 1)),
        "bf2rep": np.tile(np.asarray(inputs["bf2"], np.float32)[None, :], (128, 1)),
        "gb": np.stack([np.asarray(inputs["gamma"], np.float32),
                        np.asarray(inputs["beta"], np.float32)]),
        "ident": bf(np.eye(128, dtype=np.float32)),
        "ones": np.ones((128, 128), np.float32),
    }
    in_maps = []
    for c in range(NCORES):
        m = dict(common)
        m["xT"] = np.ascontiguousarray(bf(x[c * NLOC:(c + 1) * NLOC].T))
        m["src_tab"] = src_tabs[c]
        m["dst_tab"] = dst_tabs[c]
        m["mask"] = masks[c]
        in_maps.append(m)
    return in_maps


def kernel(**inputs):
    ei = np.asarray(inputs["edge_index"])
    nch, ch_off, epad, src_tabs, dst_tabs, masks = _prep_graph(ei)
    in_maps = _make_in_maps(inputs, src_tabs, dst_tabs, masks)
    nc = build(nch, ch_off)
    res = run_bass_kernel_spmd(nc, in_maps, list(range(NCORES)))
    out = np.concatenate([res.results[c]["out"] for c in range(NCORES)], 0)
    return out.astype(np.float32)


if __name__ == "__main__":
    ei = np.random.default_rng(0).integers(0, N, (2, E))
    print("prep ok:", _prep_graph(ei)[0])
